# revision 30
# baseline (speedup 1.0000x reference)
"""Trainium2 Bass kernel for nn_CNNGNN (CNN head + 3-layer GAT + cross-seq MHA).

Sharding: data-parallel over the B=8 sequence dim (one sequence per core).
Per core: spatial-pool 1/8 of feats (the memory-bound phase, streamed in bf16
with the reduction overlapped under the DMA), attention-gate + projection MLP,
3 GAT layers in a dense T x T formulation, and the per-sequence pooling/conv
head. Each core returns its 512-dim sequence vector; the tiny cross-sequence
MHA + classifier (0.0004% of the FLOPs, 16 KB of data) runs as a host
epilogue on the gathered [8, 512] matrix -- an on-device AllGather costs a
fixed ~90us on this stack, dwarfing the actual math.

Self-contained: hardcodes shapes from the problem spec; builds/compiles the
Bass program on first call and runs it on cores 0-7 via run_bass_kernel_spmd.
"""

import numpy as np
import ml_dtypes

import concourse.bacc as bacc
import concourse.mybir as mybir
from concourse import tile, bass_utils

F32 = mybir.dt.float32
BF16 = mybir.dt.bfloat16
AX = mybir.AxisListType
ACTF = mybir.ActivationFunctionType
ALU = mybir.AluOpType

N_CORES = 8
B, T, CB, S = 8, 128, 1280, 49          # sequences, nodes/seq, channels, spatial
FFLAT = CB * S                          # 62720
HD, NH, DH, L = 256, 8, 32, 3           # GAT hidden, heads, head dim, layers
NCH, CBLK = 20, 64                      # feats chunking: 20 chunks x 64 channels

DEBUG = False


class _StopEmit(Exception):
    pass


def _bf(x):
    return np.asarray(x, dtype=ml_dtypes.bfloat16)


def _f32(x):
    return np.ascontiguousarray(np.asarray(x, dtype=np.float32))


# ---------------------------------------------------------------------------
# Weight blob layout (shared between host prep and program builder).
# Each entry: (name, rows, cols). Offsets in columns, 8-col aligned.
# ---------------------------------------------------------------------------

def _mk_layout(entries):
    off, lay = 0, {}
    for name, rows, cols in entries:
        lay[name] = (off, rows, cols)
        off += (cols + 7) & ~7
    return lay, off


_BF_ENTRIES = [
    ("ident_b", 128, 128),
    ("blockones", 8, 1024),
    ("allones8", 8, 128),
    ("attw1T", 128, 10 * 80),
    ("attw2T", 80, 1280),
    ("projw1T", 128, 10 * 1024),
    ("projw2T", 128, 8 * 512),
    ("inwT", 128, 4 * 256),
    ("tpwT", 128, 4 * 256),
    ("c1wT", 128, 2 * 256),
    ("c2wT", 128, 2 * 256),
] + [(f"{n}{l}", 128, c) for l in range(L)
     for n, c in (("gatwT", 2 * 256), ("reswT", 2 * 256), ("asm", 2 * 8), ("adm", 2 * 8))]

_F_ENTRIES = [
    ("ident_f", 128, 128),
    ("rep_f", 8, 256),
    ("onescol_f", 128, 8),      # only col 0 used
    ("onesrow_f", 1, 128),
    ("attb1", 80, 8),
    ("attb2", 128, 10),
    ("projb1", 128, 8),
    ("projb2", 128, 8),
    ("inb", 128, 8),
    ("tpb", 128, 8),
    ("c1s", 128, 8), ("c1t", 128, 8), ("c2s", 128, 8), ("c2t", 128, 8),
    ("cnt_sd", 128, 128),
] + [(f"{n}{l}", 128, 8) for l in range(L)
     for n in ("gatb", "resb", "lng", "lnb")]

BF_LAYOUT, BF_COLS = _mk_layout(_BF_ENTRIES)
F_LAYOUT, F_COLS = _mk_layout(_F_ENTRIES)


def _pack(layout, total_cols, tensors, np_dtype):
    blob = np.zeros((128, total_cols), np_dtype)
    for name, arr in tensors.items():
        off, rows, cols = layout[name]
        a = np.asarray(arr)
        blob[:a.shape[0], off:off + a.shape[1]] = a
    return blob


# ---------------------------------------------------------------------------
# Host-side input prep
# ---------------------------------------------------------------------------

def _ktile(w):
    # [K, M] -> [128, (K/128)*M] with k-tile blocks side by side
    K, M = w.shape
    return np.concatenate([w[k * 128:(k + 1) * 128, :] for k in range(K // 128)],
                          axis=1)


def prep_in_maps(feats, edge_index, params):
    p = {k: _f32(v) for k, v in params.items()}
    feats = _bf(np.asarray(feats, np.float32).reshape(B, T, FFLAT))
    edge_index = np.asarray(edge_index)
    eps = 1e-5

    bft = {}
    bft["ident_b"] = np.eye(128, dtype=np.float32)
    blockones = np.zeros((8, NH * 128), np.float32)
    for h in range(NH):
        blockones[h, h * 128:(h + 1) * 128] = 1.0
    bft["blockones"] = blockones
    bft["allones8"] = np.ones((8, 128), np.float32)
    bft["attw1T"] = _ktile(p["att_w1"].T)
    bft["attw2T"] = p["att_w2"].T
    bft["projw1T"] = _ktile(p["proj_w1"].T)
    bft["projw2T"] = _ktile(p["proj_w2"].T)
    bft["inwT"] = _ktile(p["in_w"].T)
    bft["tpwT"] = _ktile(p["tp_w"].T)
    bft["c1wT"] = _ktile(p["c1_w"][:, :, 1].T)
    bft["c2wT"] = _ktile(p["c2_w"][:, :, 2].T)
    for l in range(L):
        bft[f"gatwT{l}"] = _ktile(p["gat_w"][l].T)
        bft[f"reswT{l}"] = _ktile(p["res_w"][l].T)
        asv = np.zeros((HD, NH), np.float32)
        adv = np.zeros((HD, NH), np.float32)
        for h in range(NH):
            asv[h * DH:(h + 1) * DH, h] = p["gat_as"][l, h]
            adv[h * DH:(h + 1) * DH, h] = p["gat_ad"][l, h]
        bft[f"asm{l}"] = _ktile(asv)
        bft[f"adm{l}"] = _ktile(adv)
    wblob = _pack(BF_LAYOUT, BF_COLS, {k: _bf(v) for k, v in bft.items()},
                  ml_dtypes.bfloat16)

    ft = {}
    ft["ident_f"] = np.eye(128, dtype=np.float32)
    rep = np.zeros((8, HD), np.float32)
    for h in range(NH):
        rep[h, h * DH:(h + 1) * DH] = 1.0
    ft["rep_f"] = rep
    ft["onescol_f"] = np.ones((128, 1), np.float32)
    ft["onesrow_f"] = np.ones((1, 128), np.float32)
    ft["attb1"] = p["att_b1"][:, None]
    ft["attb2"] = p["att_b2"].reshape(10, 128).T
    ft["projb1"] = p["proj_b1"].reshape(8, 128).T
    ft["projb2"] = p["proj_b2"].reshape(4, 128).T
    ft["inb"] = p["in_b"].reshape(2, 128).T
    ft["tpb"] = p["tp_b"].reshape(2, 128).T
    s1 = p["bn1_g"] / np.sqrt(p["bn1_v"] + eps)
    ft["c1s"] = s1.reshape(2, 128).T
    ft["c1t"] = ((p["c1_b"] - p["bn1_m"]) * s1 + p["bn1_b"]).reshape(2, 128).T
    s2 = p["bn2_g"] / np.sqrt(p["bn2_v"] + eps)
    ft["c2s"] = s2.reshape(2, 128).T
    ft["c2t"] = ((p["c2_b"] - p["bn2_m"]) * s2 + p["bn2_b"]).reshape(2, 128).T
    for l in range(L):
        ft[f"gatb{l}"] = p["gat_b"][l].reshape(2, 128).T
        ft[f"resb{l}"] = p["res_b"][l].reshape(2, 128).T
        ft[f"lng{l}"] = p["ln_g"][l].reshape(2, 128).T
        ft[f"lnb{l}"] = p["ln_b"][l].reshape(2, 128).T

    in_maps = []
    for b in range(B):
        cnt = np.zeros((T, T), np.float32)      # cnt[src, dst]
        src = edge_index[b, 0].astype(np.int64)
        dst = edge_index[b, 1].astype(np.int64)
        np.add.at(cnt, (src, dst), 1.0)
        cnt[np.arange(T), np.arange(T)] += 1.0  # self loops
        fb = dict(ft)
        fb["cnt_sd"] = cnt
        in_maps.append({
            "feats": feats[b],
            "wblob": wblob,
            "fblob": _pack(F_LAYOUT, F_COLS, fb, np.float32),
        })
    return in_maps


def host_epilogue(comb, p):
    """Cross-sequence MHA + classifier on the gathered [8, 512] matrix."""
    comb = np.asarray(comb, np.float32)
    qkv = comb @ _f32(p["mha_in_w"]).T + _f32(p["mha_in_b"])
    q, k, v = np.split(qkv, 3, -1)
    q = q.reshape(B, 8, 64)
    k = k.reshape(B, 8, 64)
    v = v.reshape(B, 8, 64)
    aw = np.einsum("qhd,khd->hqk", q, k) / 8.0
    aw = np.exp(aw - aw.max(-1, keepdims=True))
    aw = aw / aw.sum(-1, keepdims=True)
    ao = np.einsum("hqk,khd->qhd", aw, v).reshape(B, 512)
    ao = ao @ _f32(p["mha_out_w"]).T + _f32(p["mha_out_b"])
    z1 = np.maximum(ao @ _f32(p["cls_w1"]).T + _f32(p["cls_b1"]), 0)
    z2 = np.maximum(z1 @ _f32(p["cls_w2"]).T + _f32(p["cls_b2"]), 0)
    return z2 @ _f32(p["cls_w3"]).T + _f32(p["cls_b3"])


# ---------------------------------------------------------------------------
# Bass program
# ---------------------------------------------------------------------------

def build_program(debug=DEBUG, stop_after=None):
    nc = bacc.Bacc("TRN2", target_bir_lowering=False, debug=False,
                   num_devices=N_CORES)
    feats_d = nc.declare_dram_parameter("feats", [T, FFLAT], BF16, isOutput=False)
    wblob_d = nc.declare_dram_parameter("wblob", [128, BF_COLS], BF16, isOutput=False)
    fblob_d = nc.declare_dram_parameter("fblob", [128, F_COLS], F32, isOutput=False)
    comb_d = nc.dram_tensor("comb", [1, 512], F32, kind="ExternalOutput")
    dbg = {}
    if debug:
        dbg["pooled"] = nc.dram_tensor("dbg_pooled", [T, CB], F32,
                                       kind="ExternalOutput")
        dbg["h0"] = nc.dram_tensor("dbg_h0", [2, 128, 128], F32,
                                   kind="ExternalOutput")
        for l in range(L):
            dbg[f"hl{l}"] = nc.dram_tensor(f"dbg_hl{l}", [2, 128, 128], F32,
                                           kind="ExternalOutput")
    with tile.TileContext(nc) as tc:
        _emit(nc, tc, feats_d, wblob_d, fblob_d, comb_d, dbg, stop_after)
    nc.compile()
    return nc


def _emit(nc, tc, feats_d, wblob_d, fblob_d, comb_d, dbg, stop_after=None):
    wp = tc.alloc_tile_pool(name="weights", bufs=1)
    act = tc.alloc_tile_pool(name="acts", bufs=1)

    wblob = wp.tile([128, BF_COLS], BF16, tag="wblob")
    fblob = wp.tile([128, F_COLS], F32, tag="fblob")

    def W(name, rslice=None):
        off, rows, cols = BF_LAYOUT[name]
        return wblob[0:(rslice or rows), off:off + cols]

    def F(name, rows=None, cols=None):
        off, r, c = F_LAYOUT[name]
        return fblob[0:(rows or r), off:off + (cols or c)]

    try:

        eps_t = act.tile([1, 1], F32, tag="eps_t")
        nc.vector.memset(eps_t[:, :], 1e-5)
        warm_t = act.tile([1, 1], F32, tag="warm_t")
        onesc = act.tile([128, 1], BF16, tag="onesc")
        nc.vector.memset(onesc[:, :], 1.0)

        # ============ Phase A: spatial pooling (DMA + DVE overlap) =======
        # Two HWDGE queues (sync + scalar), byte-balanced. wblob is split:
        # the early columns (identity/attention weights) land first; the big
        # projection-weight tail is halved across both queues mid-stream.
        wsplit = BF_LAYOUT["projw1T"][0]
        whalf = wsplit + ((BF_COLS - wsplit) // 2 + 7) & ~7
        pooled_nm = act.tile([128, CB], F32, tag="pooled_nm")
        with tc.tile_pool(name="fchunk", bufs=4) as fpool, \
             tc.tile_pool(name="gscr", bufs=2) as gpool:
            for ch in range(NCH):
                ft = fpool.tile([128, CBLK * S], BF16, tag="f")
                eng = nc.sync if ch % 2 == 0 else nc.scalar
                eng.dma_start(out=ft[:, :],
                              in_=feats_d.ap()[:, ch * CBLK * S:(ch + 1) * CBLK * S])
                if ch == 0:
                    nc.scalar.dma_start(out=fblob[:, :], in_=fblob_d.ap())
                    nc.sync.dma_start(out=wblob[:, 0:wsplit],
                                      in_=wblob_d.ap()[:, 0:wsplit])
                elif ch == 9:
                    nc.sync.dma_start(out=wblob[:, wsplit:whalf],
                                      in_=wblob_d.ap()[:, wsplit:whalf])
                    nc.scalar.dma_start(out=wblob[:, whalf:BF_COLS],
                                        in_=wblob_d.ap()[:, whalf:BF_COLS])
                out_sl = pooled_nm[:, ch * CBLK:(ch + 1) * CBLK]
                if ch % 4 == 3:
                    # pairwise tree on GpSimd to offload the Vector engine
                    fv = ft.rearrange("p (c s) -> p c s", s=S)
                    g1 = gpool.tile([128, CBLK, 24], F32, tag="g1")
                    nc.gpsimd.tensor_add(out=g1[:, :, :], in0=fv[:, :, 0:24],
                                         in1=fv[:, :, 24:48])
                    g2 = gpool.tile([128, CBLK, 12], F32, tag="g2")
                    nc.gpsimd.tensor_add(out=g2[:, :, :], in0=g1[:, :, 0:12],
                                         in1=g1[:, :, 12:24])
                    g3 = gpool.tile([128, CBLK, 6], F32, tag="g3")
                    nc.gpsimd.tensor_add(out=g3[:, :, :], in0=g2[:, :, 0:6],
                                         in1=g2[:, :, 6:12])
                    g4 = gpool.tile([128, CBLK, 3], F32, tag="g4")
                    nc.gpsimd.tensor_add(out=g4[:, :, :], in0=g3[:, :, 0:3],
                                         in1=g3[:, :, 3:6])
                    g5 = gpool.tile([128, CBLK, 1], F32, tag="g5")
                    nc.gpsimd.tensor_add(out=g5[:, :, :], in0=g4[:, :, 0:1],
                                         in1=g4[:, :, 1:2])
                    g6 = gpool.tile([128, CBLK, 1], F32, tag="g6")
                    nc.gpsimd.tensor_add(out=g6[:, :, :], in0=g5[:, :, :],
                                         in1=g4[:, :, 2:3])
                    nc.gpsimd.tensor_add(
                        out=out_sl.unsqueeze(2),
                        in0=g6[:, :, :],
                        in1=fv[:, :, 48:49])
                else:
                    nc.vector.reduce_sum(
                        out=out_sl, in_=ft.rearrange("p (c s) -> p c s", s=S),
                        axis=AX.X)
        if dbg:
            nc.sync.dma_start(out=dbg["pooled"].ap(), in_=pooled_nm[:, :])

        poolbf = act.tile([128, CB], BF16, tag="poolbf")
        gated = act.tile([128, CB], BF16, tag="gated")

        with tc.tile_pool(name="psA", bufs=1, space="PSUM") as psA, \
             tc.tile_pool(name="sbA", bufs=3) as sbA:
            for m in range(10):
                tp = psA.tile([128, 128], F32, tag="ps", bufs=4)
                nc.tensor.transpose(tp[:, :], pooled_nm[:, m * 128:(m + 1) * 128],
                                    F("ident_f"))
                nc.scalar.activation(out=poolbf[:, m * 128:(m + 1) * 128],
                                     in_=tp[:, :], func=ACTF.Identity, scale=1.0 / S)
            atth_ps = psA.tile([80, 128], F32, tag="atth", bufs=1)
            for m in range(10):
                nc.tensor.matmul(atth_ps[:, :], W("attw1T")[:, m * 80:m * 80 + 80],
                                 poolbf[:, m * 128:(m + 1) * 128],
                                 start=(m == 0), stop=(m == 9))
            atth = sbA.tile([80, 128], BF16, tag="atthb")
            nc.scalar.activation(out=atth[:, :], in_=atth_ps[:, :], func=ACTF.Relu,
                                 bias=F("attb1", cols=1))
            for m in range(10):
                aps = psA.tile([128, 128], F32, tag="ps", bufs=4)
                nc.tensor.matmul(aps[:, :], W("attw2T")[:, m * 128:(m + 1) * 128],
                                 atth[:, :], start=True, stop=True)
                attb = sbA.tile([128, 128], BF16, tag="attm")
                nc.scalar.activation(out=attb[:, :], in_=aps[:, :], func=ACTF.Sigmoid,
                                     bias=F("attb2")[:, m:m + 1])
                nc.vector.tensor_mul(out=gated[:, m * 128:(m + 1) * 128],
                                     in0=poolbf[:, m * 128:(m + 1) * 128],
                                     in1=attb[:, :])

            # ============ Phase B: projection MLP ========================
            x1 = act.tile([128, 1024], BF16, tag="x1")
            for m in range(8):
                ps = psA.tile([128, 128], F32, tag="ps", bufs=4)
                for k in range(10):
                    nc.tensor.matmul(
                        ps[:, :],
                        W("projw1T")[:, k * 1024 + m * 128: k * 1024 + (m + 1) * 128],
                        gated[:, k * 128:(k + 1) * 128],
                        start=(k == 0), stop=(k == 9))
                nc.scalar.activation(out=x1[:, m * 128:(m + 1) * 128], in_=ps[:, :],
                                     func=ACTF.Relu, bias=F("projb1")[:, m:m + 1])
            x2 = act.tile([128, 512], BF16, tag="x2")
            for m in range(4):
                ps = psA.tile([128, 128], F32, tag="ps", bufs=4)
                for k in range(8):
                    nc.tensor.matmul(
                        ps[:, :],
                        W("projw2T")[:, k * 512 + m * 128: k * 512 + (m + 1) * 128],
                        x1[:, k * 128:(k + 1) * 128],
                        start=(k == 0), stop=(k == 7))
                nc.scalar.activation(out=x2[:, m * 128:(m + 1) * 128], in_=ps[:, :],
                                     func=ACTF.Relu, bias=F("projb2")[:, m:m + 1])
            hb = act.tile([128, 256], BF16, tag="hb")
            for m in range(2):
                ps = psA.tile([128, 128], F32, tag="ps", bufs=4)
                for k in range(4):
                    nc.tensor.matmul(
                        ps[:, :],
                        W("inwT")[:, k * 256 + m * 128: k * 256 + (m + 1) * 128],
                        x2[:, k * 128:(k + 1) * 128],
                        start=(k == 0), stop=(k == 3))
                nc.scalar.activation(out=hb[:, m * 128:(m + 1) * 128], in_=ps[:, :],
                                     func=ACTF.Identity, bias=F("inb")[:, m:m + 1])
            if dbg:
                h0d = sbA.tile([128, 128], F32, tag="h0d")
                for m in range(2):
                    nc.scalar.copy(out=h0d[:, :], in_=hb[:, m * 128:(m + 1) * 128])
                    nc.sync.dma_start(out=dbg["h0"].ap()[m], in_=h0d[:, :])

        if stop_after == "B":
            raise _StopEmit

        # ============ Phase C: GAT layers ================================
        h3f = act.tile([128, 256], F32, tag="h3f")

        with tc.tile_pool(name="psD", bufs=1, space="PSUM") as psD, \
             tc.tile_pool(name="psS", bufs=1, space="PSUM") as psS, \
             tc.tile_pool(name="sbC", bufs=2) as sbC:
            for l in range(L):
                last = (l == L - 1)
                zb = sbC.tile([128, 256], BF16, tag="zb")
                hres = sbC.tile([128, 256], F32, tag="hres")
                for m in range(2):
                    ps = psS.tile([128, 128], F32, tag="ps", bufs=2)
                    for k in range(2):
                        nc.tensor.matmul(
                            ps[:, :],
                            W(f"gatwT{l}")[:, k * 256 + m * 128: k * 256 + (m + 1) * 128],
                            hb[:, k * 128:(k + 1) * 128],
                            start=(k == 0), stop=(k == 1))
                    nc.scalar.copy(out=zb[:, m * 128:(m + 1) * 128], in_=ps[:, :])
                    ps2 = psS.tile([128, 128], F32, tag="ps", bufs=2)
                    for k in range(2):
                        nc.tensor.matmul(
                            ps2[:, :],
                            W(f"reswT{l}")[:, k * 256 + m * 128: k * 256 + (m + 1) * 128],
                            hb[:, k * 128:(k + 1) * 128],
                            start=(k == 0), stop=(k == 1))
                    nc.scalar.activation(out=hres[:, m * 128:(m + 1) * 128],
                                         in_=ps2[:, :], func=ACTF.Identity,
                                         bias=F(f"resb{l}")[:, m:m + 1])
                # node attention scores per head
                esed = psS.tile([8, 256], F32, tag="esed", bufs=1)
                for k in range(2):
                    nc.tensor.matmul(esed[:, 0:128],
                                     W(f"asm{l}")[:, k * 8:(k + 1) * 8],
                                     zb[:, k * 128:(k + 1) * 128],
                                     start=(k == 0), stop=(k == 1))
                for k in range(2):
                    nc.tensor.matmul(esed[:, 128:256],
                                     W(f"adm{l}")[:, k * 8:(k + 1) * 8],
                                     zb[:, k * 128:(k + 1) * 128],
                                     start=(k == 0), stop=(k == 1))
                es_sb = sbC.tile([8, 128], BF16, tag="essb")
                ed_sb = sbC.tile([8, 128], BF16, tag="essb")
                nc.scalar.copy(out=es_sb[:, :], in_=esed[:, 0:128])
                nc.scalar.copy(out=ed_sb[:, :], in_=esed[:, 128:256])

                # dense1[s, (h,d)] = es[h,s] + ed[h,d]:
                #   es part: blockones spreads es rows across column blocks;
                #   ed part: blockones block h is the row-h selector, so
                #   sel_h.T @ ed broadcasts ed row h down all partitions.
                dense1 = psD.tile([128, 1024], F32, tag="dense")
                for h in range(NH):
                    sl = slice(h * 128, (h + 1) * 128)
                    nc.tensor.matmul(dense1[:, sl], es_sb[:, :],
                                     W("blockones")[:, sl], start=True, stop=False)
                    nc.tensor.matmul(dense1[:, sl], W("blockones")[:, sl],
                                     ed_sb[:, :], start=False, stop=True)
                # warm the Exp/Sqrt activation tables off the critical chain
                nc.scalar.activation(out=warm_t[:, :], in_=eps_t[:, :],
                                     func=ACTF.Exp)
                # leaky_relu(x) = max(x, 0.2x), then exp, then * cnt
                # (processed in 512-col halves so ACT/DVE pipeline)
                d1c = sbC.tile([128, 1024], F32, tag="d1c", bufs=1)
                e1 = sbC.tile([128, 1024], F32, tag="e1", bufs=1)
                w1 = sbC.tile([128, 1024], F32, tag="w1", bufs=1)
                wc = sbC.tile([128, 1024], BF16, tag="wc", bufs=1)
                den_ps = psD.tile([1, 1024], F32, tag="dense", bufs=1, name="den_ps")
                for nchunk in range(2):
                    sl = slice(nchunk * 512, (nchunk + 1) * 512)
                    nc.scalar.activation(out=d1c[:, sl], in_=dense1[:, sl],
                                         func=ACTF.Identity)
                    nc.vector.scalar_tensor_tensor(out=e1[:, sl], in0=d1c[:, sl],
                                                   scalar=0.2, in1=d1c[:, sl],
                                                   op0=ALU.mult, op1=ALU.max)
                    nc.scalar.activation(out=w1[:, sl], in_=e1[:, sl],
                                         func=ACTF.Exp)
                    nc.vector.tensor_mul(
                        out=wc[:, sl].rearrange("p (h d) -> p h d", h=4),
                        in0=w1[:, sl].rearrange("p (h d) -> p h d", h=4),
                        in1=F("cnt_sd").unsqueeze(1).broadcast_to([128, 4, 128]))
                    nc.tensor.matmul(den_ps[:, sl], onesc[:, :], wc[:, sl],
                                     start=True, stop=True)
                nc.scalar.activation(out=warm_t[:, :], in_=eps_t[:, :],
                                     func=ACTF.Sqrt)
                den_row = sbC.tile([1, 1024], F32, tag="den_row")
                nc.scalar.copy(out=den_row[:, :], in_=den_ps[:, :])
                dent_ps = psS.tile([128, 8], F32, tag="rdt", bufs=1)
                for h in range(NH):
                    nc.tensor.transpose(dent_ps[:, h:h + 1],
                                        den_row[0:1, h * 128:(h + 1) * 128],
                                        F("ident_f", rows=1, cols=1))
                den_dh = sbC.tile([128, 8], F32, tag="den_dh")
                nc.scalar.copy(out=den_dh[:, :], in_=dent_ps[:, :])
                rden = sbC.tile([128, 8], F32, tag="rden")
                nc.vector.reciprocal(out=rden[:, :], in_=den_dh[:, :])
                rdt_ps = psS.tile([8, 128], F32, tag="rdt", bufs=1)
                nc.tensor.transpose(rdt_ps[:, :], rden[:, :], F("ident_f"))
                rdt = sbC.tile([8, 128], F32, tag="rdts")
                nc.scalar.copy(out=rdt[:, :], in_=rdt_ps[:, :])
                denx = sbC.tile([128, 256], F32, tag="denx")
                for m in range(2):
                    dps = psS.tile([128, 128], F32, tag="ps", bufs=2)
                    nc.tensor.matmul(dps[:, :], F("rep_f")[:, m * 128:(m + 1) * 128],
                                     rdt[:, :], start=True, stop=True)
                    nc.scalar.copy(out=denx[:, m * 128:(m + 1) * 128], in_=dps[:, :])
                # zT + aggregation
                zT = sbC.tile([128, 256], BF16, tag="zT")
                for m in range(2):
                    tp = psS.tile([128, 128], BF16, tag="ps", bufs=2)
                    nc.tensor.transpose(tp[:, :], zb[:, m * 128:(m + 1) * 128],
                                        W("ident_b"))
                    nc.scalar.copy(out=zT[:, m * 128:(m + 1) * 128], in_=tp[:, :])
                agg = psS.tile([128, 256], F32, tag="agg", bufs=1)
                for h in range(NH):
                    nc.tensor.matmul(
                        agg[32 * (h % 4):32 * (h % 4) + 32,
                            128 * (h // 4):128 * (h // 4) + 128],
                        zT[:, h * 32:(h + 1) * 32],
                        wc[:, h * 128:(h + 1) * 128],
                        start=True, stop=True,
                        tile_position=(0, 32 * (h % 4)))
                # combine + LayerNorm + relu
                xln = sbC.tile([128, 256], F32, tag="xln")
                for m in range(2):
                    tmp = sbC.tile([128, 128], F32, tag="tmp")
                    nc.vector.tensor_mul(out=tmp[:, :],
                                         in0=agg[:, m * 128:(m + 1) * 128],
                                         in1=denx[:, m * 128:(m + 1) * 128])
                    nc.vector.scalar_tensor_tensor(
                        out=xln[:, m * 128:(m + 1) * 128], in0=tmp[:, :],
                        scalar=F(f"gatb{l}")[:, m:m + 1],
                        in1=hres[:, m * 128:(m + 1) * 128],
                        op0=ALU.add, op1=ALU.add)
                xsq = sbC.tile([128, 256], F32, tag="xsq")
                nc.vector.tensor_mul(out=xsq[:, :], in0=xln[:, :], in1=xln[:, :])
                stats = psS.tile([1, 256], F32, tag="stats", bufs=1)
                for k in range(2):
                    nc.tensor.matmul(stats[:, 0:128], F("onescol_f", cols=1),
                                     xln[:, k * 128:(k + 1) * 128],
                                     start=(k == 0), stop=(k == 1))
                for k in range(2):
                    nc.tensor.matmul(stats[:, 128:256], F("onescol_f", cols=1),
                                     xsq[:, k * 128:(k + 1) * 128],
                                     start=(k == 0), stop=(k == 1))
                stsb = sbC.tile([1, 256], F32, tag="stsb")
                nc.vector.tensor_scalar_mul(out=stsb[:, :], in0=stats[:, :],
                                            scalar1=1.0 / 256.0)
                musq = sbC.tile([1, 128], F32, tag="musq")
                nc.vector.tensor_mul(out=musq[:, :], in0=stsb[:, 0:128],
                                     in1=stsb[:, 0:128])
                var = sbC.tile([1, 128], F32, tag="var")
                nc.vector.tensor_sub(out=var[:, :], in0=stsb[:, 128:256],
                                     in1=musq[:, :])
                sd = sbC.tile([1, 128], F32, tag="sd")
                nc.scalar.activation(out=sd[:, :], in_=var[:, :], func=ACTF.Sqrt,
                                     bias=eps_t[:, :])
                rstd = sbC.tile([1, 128], F32, tag="rstd")
                nc.vector.reciprocal(out=rstd[:, :], in_=sd[:, :])
                musd = psS.tile([128, 256], F32, tag="stats", bufs=1)
                nc.tensor.matmul(musd[:, 0:128], F("onesrow_f"), stsb[:, 0:128],
                                 start=True, stop=True)
                nc.tensor.matmul(musd[:, 128:256], F("onesrow_f"), rstd[:, :],
                                 start=True, stop=True)
                hb_next = act.tile([128, 256], BF16, tag=f"hb{l + 1}",
                                   name=f"hbn{l}") if not last else None
                for m in range(2):
                    df = sbC.tile([128, 128], F32, tag="df")
                    nc.vector.tensor_sub(out=df[:, :],
                                         in0=xln[:, m * 128:(m + 1) * 128],
                                         in1=musd[:, 0:128])
                    dn = sbC.tile([128, 128], F32, tag="dn")
                    nc.vector.tensor_mul(out=dn[:, :], in0=df[:, :],
                                         in1=musd[:, 128:256])
                    dst = h3f if last else hb_next
                    nc.scalar.activation(out=dst[:, m * 128:(m + 1) * 128],
                                         in_=dn[:, :], func=ACTF.Relu,
                                         bias=F(f"lnb{l}")[:, m:m + 1],
                                         scale=F(f"lng{l}")[:, m:m + 1])
                if dbg:
                    hld = sbC.tile([128, 128], F32, tag="hld")
                    for m in range(2):
                        src = h3f if last else hb_next
                        nc.scalar.copy(out=hld[:, :],
                                       in_=src[:, m * 128:(m + 1) * 128])
                        nc.sync.dma_start(out=dbg[f"hl{l}"].ap()[m], in_=hld[:, :])
                if not last:
                    hb = hb_next

        if stop_after == "C":
            raise _StopEmit

        # ============ Phase D: pool/conv head -> comb output =============
        with tc.tile_pool(name="psE", bufs=1, space="PSUM") as psE, \
             tc.tile_pool(name="sbE", bufs=2) as sbE:
            hmean = sbE.tile([128, 2], F32, tag="hmean")
            hmax = sbE.tile([128, 2], F32, tag="hmax")
            for m in range(2):
                nc.vector.reduce_sum(out=hmean[:, m:m + 1],
                                     in_=h3f[:, m * 128:(m + 1) * 128], axis=AX.X)
                nc.vector.reduce_max(out=hmax[:, m:m + 1],
                                     in_=h3f[:, m * 128:(m + 1) * 128], axis=AX.X)
            catb = sbE.tile([128, 4], BF16, tag="catb")
            for m in range(2):
                nc.scalar.activation(out=catb[:, m:m + 1], in_=hmean[:, m:m + 1],
                                     func=ACTF.Identity, scale=1.0 / T)
                nc.scalar.copy(out=catb[:, 2 + m:3 + m], in_=hmax[:, m:m + 1])
            cat4 = sbE.tile([128, 4], F32, tag="cat4")
            tf_b = sbE.tile([128, 2], BF16, tag="tf_b")
            for m in range(2):
                ps = psE.tile([128, 1], F32, tag="v", bufs=2)
                for k in range(4):
                    nc.tensor.matmul(
                        ps[:, :],
                        W("tpwT")[:, k * 256 + m * 128: k * 256 + (m + 1) * 128],
                        catb[:, k:k + 1], start=(k == 0), stop=(k == 3))
                nc.scalar.activation(out=cat4[:, m:m + 1], in_=ps[:, :],
                                     func=ACTF.Relu, bias=F("tpb")[:, m:m + 1])
            nc.vector.tensor_copy(out=tf_b[:, :], in_=cat4[:, 0:2])
            c1_b = sbE.tile([128, 2], BF16, tag="c1_b")
            for m in range(2):
                ps = psE.tile([128, 1], F32, tag="v", bufs=2)
                for k in range(2):
                    nc.tensor.matmul(
                        ps[:, :],
                        W("c1wT")[:, k * 256 + m * 128: k * 256 + (m + 1) * 128],
                        tf_b[:, k:k + 1], start=(k == 0), stop=(k == 1))
                nc.scalar.activation(out=c1_b[:, m:m + 1], in_=ps[:, :],
                                     func=ACTF.Relu, bias=F("c1t")[:, m:m + 1],
                                     scale=F("c1s")[:, m:m + 1])
            for m in range(2):
                ps = psE.tile([128, 1], F32, tag="v", bufs=2)
                for k in range(2):
                    nc.tensor.matmul(
                        ps[:, :],
                        W("c2wT")[:, k * 256 + m * 128: k * 256 + (m + 1) * 128],
                        c1_b[:, k:k + 1], start=(k == 0), stop=(k == 1))
                nc.scalar.activation(out=cat4[:, 2 + m:3 + m], in_=ps[:, :],
                                     func=ACTF.Relu, bias=F("c2t")[:, m:m + 1],
                                     scale=F("c2s")[:, m:m + 1])
            ctp = psE.tile([4, 128], F32, tag="ctp", bufs=1)
            nc.tensor.transpose(ctp[:, :], cat4[:, :], F("ident_f"))
            comb_sb = sbE.tile([4, 128], F32, tag="comb_sb")
            nc.scalar.copy(out=comb_sb[:, :], in_=ctp[:, :])
            nc.sync.dma_start(out=comb_d.ap(), in_=comb_sb[:, :])
    except _StopEmit:
        pass
    act.release()
    wp.release()


# ---------------------------------------------------------------------------
# Entry point
# ---------------------------------------------------------------------------

_NC_CACHE = {}


def get_program(debug=False):
    key = bool(debug)
    if key not in _NC_CACHE:
        _NC_CACHE[key] = build_program(debug=key)
    return _NC_CACHE[key]


def run_device(in_maps, debug=False, **kwargs):
    nc = get_program(debug=debug)
    return bass_utils.run_bass_kernel_spmd(nc, in_maps,
                                           core_ids=list(range(N_CORES)), **kwargs)


def kernel(feats, edge_index, batch_idx, params):
    del batch_idx  # all-zero; one graph per sequence
    in_maps = prep_in_maps(feats, edge_index, params)
    res = run_device(in_maps)
    comb = np.concatenate([np.asarray(res.results[b]["comb"], np.float32)
                           for b in range(B)], axis=0)
    return host_epilogue(comb, params)


# revision 32
# speedup vs baseline: 1.0886x; 1.0886x over previous
"""Trainium2 Bass kernel for nn_CNNGNN (CNN head + 3-layer GAT + cross-seq MHA).

Sharding: data-parallel over the B=8 sequence dim (one sequence per core).
Per core: spatial-pool 1/8 of feats (the memory-bound phase, streamed in bf16
with the reduction overlapped under the DMA), attention-gate + projection MLP,
3 GAT layers in a dense T x T formulation, and the per-sequence pooling/conv
head. Each core returns its 512-dim sequence vector; the tiny cross-sequence
MHA + classifier (0.0004% of the FLOPs, 16 KB of data) runs as a host
epilogue on the gathered [8, 512] matrix -- an on-device AllGather costs a
fixed ~90us on this stack, dwarfing the actual math.

Self-contained: hardcodes shapes from the problem spec; builds/compiles the
Bass program on first call and runs it on cores 0-7 via run_bass_kernel_spmd.
"""

import numpy as np
import ml_dtypes

import concourse.bacc as bacc
import concourse.mybir as mybir
from concourse import tile, bass_utils

F32 = mybir.dt.float32
BF16 = mybir.dt.bfloat16
AX = mybir.AxisListType
ACTF = mybir.ActivationFunctionType
ALU = mybir.AluOpType

N_CORES = 8
B, T, CB, S = 8, 128, 1280, 49          # sequences, nodes/seq, channels, spatial
FFLAT = CB * S                          # 62720
HD, NH, DH, L = 256, 8, 32, 3           # GAT hidden, heads, head dim, layers
NCH, CBLK = 20, 64                      # feats chunking: 20 chunks x 64 channels

DEBUG = False


class _StopEmit(Exception):
    pass


def _bf(x):
    return np.asarray(x, dtype=ml_dtypes.bfloat16)


def _f32(x):
    return np.ascontiguousarray(np.asarray(x, dtype=np.float32))


# ---------------------------------------------------------------------------
# Weight blob layout (shared between host prep and program builder).
# Each entry: (name, rows, cols). Offsets in columns, 8-col aligned.
# ---------------------------------------------------------------------------

def _mk_layout(entries):
    off, lay = 0, {}
    for name, rows, cols in entries:
        lay[name] = (off, rows, cols)
        off += (cols + 7) & ~7
    return lay, off


_BF_ENTRIES = [
    ("ident_b", 128, 128),
    ("ones32", 128, 32),
    ("blockones", 8, 1024),
    ("allones8", 8, 128),
    ("attw1T", 128, 10 * 80),
    ("attw2T", 80, 1280),
    ("projw1T", 128, 10 * 1024),
    ("projw2T", 128, 8 * 512),
    ("inwT", 128, 4 * 256),
    ("tpwT", 128, 4 * 256),
    ("c1wT", 128, 2 * 256),
    ("c2wT", 128, 2 * 256),
] + [(f"{n}{l}", 128, c) for l in range(L)
     for n, c in (("gatwT", 2 * 256), ("reswT", 2 * 256), ("asm", 2 * 8), ("adm", 2 * 8))]

_F_ENTRIES = [
    ("ident_f", 128, 128),
    ("rep_f", 8, 256),
    ("onescol_f", 128, 8),      # only col 0 used
    ("onesrow_f", 1, 128),
    ("attb1", 80, 8),
    ("attb2", 128, 10),
    ("projb1", 128, 8),
    ("projb2", 128, 8),
    ("inb", 128, 8),
    ("tpb", 128, 8),
    ("c1s", 128, 8), ("c1t", 128, 8), ("c2s", 128, 8), ("c2t", 128, 8),
    ("cnt_sd", 128, 128),
] + [(f"{n}{l}", 128, 8) for l in range(L)
     for n in ("gatb", "resb", "lng", "lnb")]

BF_LAYOUT, BF_COLS = _mk_layout(_BF_ENTRIES)
F_LAYOUT, F_COLS = _mk_layout(_F_ENTRIES)


def _pack(layout, total_cols, tensors, np_dtype):
    blob = np.zeros((128, total_cols), np_dtype)
    for name, arr in tensors.items():
        off, rows, cols = layout[name]
        a = np.asarray(arr)
        blob[:a.shape[0], off:off + a.shape[1]] = a
    return blob


# ---------------------------------------------------------------------------
# Host-side input prep
# ---------------------------------------------------------------------------

def _ktile(w):
    # [K, M] -> [128, (K/128)*M] with k-tile blocks side by side
    K, M = w.shape
    return np.concatenate([w[k * 128:(k + 1) * 128, :] for k in range(K // 128)],
                          axis=1)


def prep_in_maps(feats, edge_index, params):
    p = {k: _f32(v) for k, v in params.items()}
    feats = _bf(np.asarray(feats, np.float32).reshape(B, T, FFLAT))
    edge_index = np.asarray(edge_index)
    eps = 1e-5

    bft = {}
    bft["ident_b"] = np.eye(128, dtype=np.float32)
    bft["ones32"] = np.ones((128, 32), np.float32)
    blockones = np.zeros((8, NH * 128), np.float32)
    for h in range(NH):
        blockones[h, h * 128:(h + 1) * 128] = 1.0
    bft["blockones"] = blockones
    bft["allones8"] = np.ones((8, 128), np.float32)
    bft["attw1T"] = _ktile(p["att_w1"].T)
    bft["attw2T"] = p["att_w2"].T
    bft["projw1T"] = _ktile(p["proj_w1"].T)
    bft["projw2T"] = _ktile(p["proj_w2"].T)
    bft["inwT"] = _ktile(p["in_w"].T)
    bft["tpwT"] = _ktile(p["tp_w"].T)
    bft["c1wT"] = _ktile(p["c1_w"][:, :, 1].T)
    bft["c2wT"] = _ktile(p["c2_w"][:, :, 2].T)
    for l in range(L):
        bft[f"gatwT{l}"] = _ktile(p["gat_w"][l].T)
        bft[f"reswT{l}"] = _ktile(p["res_w"][l].T)
        asv = np.zeros((HD, NH), np.float32)
        adv = np.zeros((HD, NH), np.float32)
        for h in range(NH):
            asv[h * DH:(h + 1) * DH, h] = p["gat_as"][l, h]
            adv[h * DH:(h + 1) * DH, h] = p["gat_ad"][l, h]
        bft[f"asm{l}"] = _ktile(asv)
        bft[f"adm{l}"] = _ktile(adv)
    wblob = _pack(BF_LAYOUT, BF_COLS, {k: _bf(v) for k, v in bft.items()},
                  ml_dtypes.bfloat16)

    ft = {}
    ft["ident_f"] = np.eye(128, dtype=np.float32)
    rep = np.zeros((8, HD), np.float32)
    for h in range(NH):
        rep[h, h * DH:(h + 1) * DH] = 1.0
    ft["rep_f"] = rep
    ft["onescol_f"] = np.ones((128, 1), np.float32)
    ft["onesrow_f"] = np.ones((1, 128), np.float32)
    ft["attb1"] = p["att_b1"][:, None]
    ft["attb2"] = p["att_b2"].reshape(10, 128).T
    ft["projb1"] = p["proj_b1"].reshape(8, 128).T
    ft["projb2"] = p["proj_b2"].reshape(4, 128).T
    ft["inb"] = p["in_b"].reshape(2, 128).T
    ft["tpb"] = p["tp_b"].reshape(2, 128).T
    s1 = p["bn1_g"] / np.sqrt(p["bn1_v"] + eps)
    ft["c1s"] = s1.reshape(2, 128).T
    ft["c1t"] = ((p["c1_b"] - p["bn1_m"]) * s1 + p["bn1_b"]).reshape(2, 128).T
    s2 = p["bn2_g"] / np.sqrt(p["bn2_v"] + eps)
    ft["c2s"] = s2.reshape(2, 128).T
    ft["c2t"] = ((p["c2_b"] - p["bn2_m"]) * s2 + p["bn2_b"]).reshape(2, 128).T
    for l in range(L):
        ft[f"gatb{l}"] = p["gat_b"][l].reshape(2, 128).T
        ft[f"resb{l}"] = p["res_b"][l].reshape(2, 128).T
        ft[f"lng{l}"] = p["ln_g"][l].reshape(2, 128).T
        ft[f"lnb{l}"] = p["ln_b"][l].reshape(2, 128).T

    in_maps = []
    for b in range(B):
        cnt = np.zeros((T, T), np.float32)      # cnt[src, dst]
        src = edge_index[b, 0].astype(np.int64)
        dst = edge_index[b, 1].astype(np.int64)
        np.add.at(cnt, (src, dst), 1.0)
        cnt[np.arange(T), np.arange(T)] += 1.0  # self loops
        fb = dict(ft)
        fb["cnt_sd"] = cnt
        in_maps.append({
            "feats": feats[b],
            "wblob": wblob,
            "fblob": _pack(F_LAYOUT, F_COLS, fb, np.float32),
        })
    return in_maps


def host_epilogue(comb, p):
    """Cross-sequence MHA + classifier on the gathered [8, 512] matrix."""
    comb = np.asarray(comb, np.float32)
    qkv = comb @ _f32(p["mha_in_w"]).T + _f32(p["mha_in_b"])
    q, k, v = np.split(qkv, 3, -1)
    q = q.reshape(B, 8, 64)
    k = k.reshape(B, 8, 64)
    v = v.reshape(B, 8, 64)
    aw = np.einsum("qhd,khd->hqk", q, k) / 8.0
    aw = np.exp(aw - aw.max(-1, keepdims=True))
    aw = aw / aw.sum(-1, keepdims=True)
    ao = np.einsum("hqk,khd->qhd", aw, v).reshape(B, 512)
    ao = ao @ _f32(p["mha_out_w"]).T + _f32(p["mha_out_b"])
    z1 = np.maximum(ao @ _f32(p["cls_w1"]).T + _f32(p["cls_b1"]), 0)
    z2 = np.maximum(z1 @ _f32(p["cls_w2"]).T + _f32(p["cls_b2"]), 0)
    return z2 @ _f32(p["cls_w3"]).T + _f32(p["cls_b3"])


# ---------------------------------------------------------------------------
# Bass program
# ---------------------------------------------------------------------------

def build_program(debug=DEBUG, stop_after=None):
    nc = bacc.Bacc("TRN2", target_bir_lowering=False, debug=False,
                   num_devices=N_CORES)
    feats_d = nc.declare_dram_parameter("feats", [T, FFLAT], BF16, isOutput=False)
    wblob_d = nc.declare_dram_parameter("wblob", [128, BF_COLS], BF16, isOutput=False)
    fblob_d = nc.declare_dram_parameter("fblob", [128, F_COLS], F32, isOutput=False)
    comb_d = nc.dram_tensor("comb", [1, 512], F32, kind="ExternalOutput")
    dbg = {}
    if debug:
        dbg["pooled"] = nc.dram_tensor("dbg_pooled", [T, CB], F32,
                                       kind="ExternalOutput")
        dbg["h0"] = nc.dram_tensor("dbg_h0", [2, 128, 128], F32,
                                   kind="ExternalOutput")
        for l in range(L):
            dbg[f"hl{l}"] = nc.dram_tensor(f"dbg_hl{l}", [2, 128, 128], F32,
                                           kind="ExternalOutput")
    with tile.TileContext(nc) as tc:
        _emit(nc, tc, feats_d, wblob_d, fblob_d, comb_d, dbg, stop_after)
    nc.compile()
    return nc


def _emit(nc, tc, feats_d, wblob_d, fblob_d, comb_d, dbg, stop_after=None):
    wp = tc.alloc_tile_pool(name="weights", bufs=1)
    act = tc.alloc_tile_pool(name="acts", bufs=1)

    wblob = wp.tile([128, BF_COLS], BF16, tag="wblob")
    fblob = wp.tile([128, F_COLS], F32, tag="fblob")

    def W(name, rslice=None):
        off, rows, cols = BF_LAYOUT[name]
        return wblob[0:(rslice or rows), off:off + cols]

    def F(name, rows=None, cols=None):
        off, r, c = F_LAYOUT[name]
        return fblob[0:(rows or r), off:off + (cols or c)]

    try:

        eps_t = act.tile([1, 1], F32, tag="eps_t")
        nc.vector.memset(eps_t[:, :], 1e-5)
        onesc = act.tile([128, 1], BF16, tag="onesc")
        nc.vector.memset(onesc[:, :], 1.0)

        # ============ Phase A: spatial pooling (DMA + DVE overlap) =======
        # Two HWDGE queues (sync + scalar), byte-balanced. wblob is split:
        # the early columns (identity/attention weights) land first; the big
        # projection-weight tail is halved across both queues mid-stream.
        wsplit = BF_LAYOUT["projw1T"][0]
        whalf = wsplit + ((BF_COLS - wsplit) // 2 + 7) & ~7
        pooled_nm = act.tile([128, CB], F32, tag="pooled_nm")
        with tc.tile_pool(name="fchunk", bufs=4) as fpool, \
             tc.tile_pool(name="gscr", bufs=2) as gpool:
            for ch in range(NCH):
                ft = fpool.tile([128, CBLK * S], BF16, tag="f")
                eng = nc.sync if ch % 2 == 0 else nc.scalar
                eng.dma_start(out=ft[:, :],
                              in_=feats_d.ap()[:, ch * CBLK * S:(ch + 1) * CBLK * S])
                if ch == 0:
                    nc.scalar.dma_start(out=fblob[:, :], in_=fblob_d.ap())
                    nc.sync.dma_start(out=wblob[:, 0:wsplit],
                                      in_=wblob_d.ap()[:, 0:wsplit])
                elif ch == 9:
                    nc.sync.dma_start(out=wblob[:, wsplit:whalf],
                                      in_=wblob_d.ap()[:, wsplit:whalf])
                    nc.scalar.dma_start(out=wblob[:, whalf:BF_COLS],
                                        in_=wblob_d.ap()[:, whalf:BF_COLS])
                out_sl = pooled_nm[:, ch * CBLK:(ch + 1) * CBLK]
                if ch % 4 == 3:
                    # pairwise tree on GpSimd to offload the Vector engine
                    fv = ft.rearrange("p (c s) -> p c s", s=S)
                    g1 = gpool.tile([128, CBLK, 24], F32, tag="g1")
                    nc.gpsimd.tensor_add(out=g1[:, :, :], in0=fv[:, :, 0:24],
                                         in1=fv[:, :, 24:48])
                    g2 = gpool.tile([128, CBLK, 12], F32, tag="g2")
                    nc.gpsimd.tensor_add(out=g2[:, :, :], in0=g1[:, :, 0:12],
                                         in1=g1[:, :, 12:24])
                    g3 = gpool.tile([128, CBLK, 6], F32, tag="g3")
                    nc.gpsimd.tensor_add(out=g3[:, :, :], in0=g2[:, :, 0:6],
                                         in1=g2[:, :, 6:12])
                    g4 = gpool.tile([128, CBLK, 3], F32, tag="g4")
                    nc.gpsimd.tensor_add(out=g4[:, :, :], in0=g3[:, :, 0:3],
                                         in1=g3[:, :, 3:6])
                    g5 = gpool.tile([128, CBLK, 1], F32, tag="g5")
                    nc.gpsimd.tensor_add(out=g5[:, :, :], in0=g4[:, :, 0:1],
                                         in1=g4[:, :, 1:2])
                    g6 = gpool.tile([128, CBLK, 1], F32, tag="g6")
                    nc.gpsimd.tensor_add(out=g6[:, :, :], in0=g5[:, :, :],
                                         in1=g4[:, :, 2:3])
                    nc.gpsimd.tensor_add(
                        out=out_sl.unsqueeze(2),
                        in0=g6[:, :, :],
                        in1=fv[:, :, 48:49])
                else:
                    nc.vector.reduce_sum(
                        out=out_sl, in_=ft.rearrange("p (c s) -> p c s", s=S),
                        axis=AX.X)
        if dbg:
            nc.sync.dma_start(out=dbg["pooled"].ap(), in_=pooled_nm[:, :])

        poolbf = act.tile([128, CB], BF16, tag="poolbf")
        gated = act.tile([128, CB], BF16, tag="gated")

        with tc.tile_pool(name="psA", bufs=1, space="PSUM") as psA, \
             tc.tile_pool(name="sbA", bufs=3) as sbA:
            for m in range(10):
                tp = psA.tile([128, 128], F32, tag="ps", bufs=4)
                nc.tensor.transpose(tp[:, :], pooled_nm[:, m * 128:(m + 1) * 128],
                                    F("ident_f"))
                nc.scalar.activation(out=poolbf[:, m * 128:(m + 1) * 128],
                                     in_=tp[:, :], func=ACTF.Identity, scale=1.0 / S)
            atth_ps = psA.tile([80, 128], F32, tag="atth", bufs=1)
            for m in range(10):
                nc.tensor.matmul(atth_ps[:, :], W("attw1T")[:, m * 80:m * 80 + 80],
                                 poolbf[:, m * 128:(m + 1) * 128],
                                 start=(m == 0), stop=(m == 9))
            atth = sbA.tile([80, 128], BF16, tag="atthb")
            nc.scalar.activation(out=atth[:, :], in_=atth_ps[:, :], func=ACTF.Relu,
                                 bias=F("attb1", cols=1))
            for m in range(10):
                aps = psA.tile([128, 128], F32, tag="ps", bufs=4)
                nc.tensor.matmul(aps[:, :], W("attw2T")[:, m * 128:(m + 1) * 128],
                                 atth[:, :], start=True, stop=True)
                attb = sbA.tile([128, 128], BF16, tag="attm")
                nc.scalar.activation(out=attb[:, :], in_=aps[:, :], func=ACTF.Sigmoid,
                                     bias=F("attb2")[:, m:m + 1])
                nc.gpsimd.tensor_mul(out=gated[:, m * 128:(m + 1) * 128],
                                      in0=poolbf[:, m * 128:(m + 1) * 128],
                                      in1=attb[:, :])

            # ============ Phase B: projection MLP ========================
            x1 = act.tile([128, 1024], BF16, tag="x1")
            for m in range(8):
                ps = psA.tile([128, 128], F32, tag="ps", bufs=4)
                for k in range(10):
                    nc.tensor.matmul(
                        ps[:, :],
                        W("projw1T")[:, k * 1024 + m * 128: k * 1024 + (m + 1) * 128],
                        gated[:, k * 128:(k + 1) * 128],
                        start=(k == 0), stop=(k == 9))
                nc.scalar.activation(out=x1[:, m * 128:(m + 1) * 128], in_=ps[:, :],
                                     func=ACTF.Relu, bias=F("projb1")[:, m:m + 1])
            x2 = act.tile([128, 512], BF16, tag="x2")
            for m in range(4):
                ps = psA.tile([128, 128], F32, tag="ps", bufs=4)
                for k in range(8):
                    nc.tensor.matmul(
                        ps[:, :],
                        W("projw2T")[:, k * 512 + m * 128: k * 512 + (m + 1) * 128],
                        x1[:, k * 128:(k + 1) * 128],
                        start=(k == 0), stop=(k == 7))
                nc.scalar.activation(out=x2[:, m * 128:(m + 1) * 128], in_=ps[:, :],
                                     func=ACTF.Relu, bias=F("projb2")[:, m:m + 1])
            hb = act.tile([128, 256], BF16, tag="hb")
            for m in range(2):
                ps = psA.tile([128, 128], F32, tag="ps", bufs=4)
                for k in range(4):
                    nc.tensor.matmul(
                        ps[:, :],
                        W("inwT")[:, k * 256 + m * 128: k * 256 + (m + 1) * 128],
                        x2[:, k * 128:(k + 1) * 128],
                        start=(k == 0), stop=(k == 3))
                nc.scalar.activation(out=hb[:, m * 128:(m + 1) * 128], in_=ps[:, :],
                                     func=ACTF.Identity, bias=F("inb")[:, m:m + 1])
            if dbg:
                h0d = sbA.tile([128, 128], F32, tag="h0d")
                for m in range(2):
                    nc.scalar.copy(out=h0d[:, :], in_=hb[:, m * 128:(m + 1) * 128])
                    nc.sync.dma_start(out=dbg["h0"].ap()[m], in_=h0d[:, :])

        if stop_after == "B":
            raise _StopEmit

        # ============ Phase C: GAT layers ================================
        h3f = act.tile([128, 256], F32, tag="h3f")

        with tc.tile_pool(name="psD", bufs=1, space="PSUM") as psD, \
             tc.tile_pool(name="psS", bufs=1, space="PSUM") as psS, \
             tc.tile_pool(name="sbC", bufs=2) as sbC:
            for l in range(L):
                last = (l == L - 1)
                zb = sbC.tile([128, 256], BF16, tag="zb")
                hres = sbC.tile([128, 256], F32, tag="hres")
                for m in range(2):
                    ps = psS.tile([128, 128], F32, tag="ps", bufs=2)
                    for k in range(2):
                        nc.tensor.matmul(
                            ps[:, :],
                            W(f"gatwT{l}")[:, k * 256 + m * 128: k * 256 + (m + 1) * 128],
                            hb[:, k * 128:(k + 1) * 128],
                            start=(k == 0), stop=(k == 1))
                    nc.scalar.copy(out=zb[:, m * 128:(m + 1) * 128], in_=ps[:, :])
                    ps2 = psS.tile([128, 128], F32, tag="ps", bufs=2)
                    for k in range(2):
                        nc.tensor.matmul(
                            ps2[:, :],
                            W(f"reswT{l}")[:, k * 256 + m * 128: k * 256 + (m + 1) * 128],
                            hb[:, k * 128:(k + 1) * 128],
                            start=(k == 0), stop=(k == 1))
                    nc.scalar.activation(out=hres[:, m * 128:(m + 1) * 128],
                                         in_=ps2[:, :], func=ACTF.Identity,
                                         bias=F(f"resb{l}")[:, m:m + 1])
                # node attention scores per head
                esed = psS.tile([8, 256], F32, tag="esed", bufs=1)
                for k in range(2):
                    nc.tensor.matmul(esed[:, 0:128],
                                     W(f"asm{l}")[:, k * 8:(k + 1) * 8],
                                     zb[:, k * 128:(k + 1) * 128],
                                     start=(k == 0), stop=(k == 1))
                for k in range(2):
                    nc.tensor.matmul(esed[:, 128:256],
                                     W(f"adm{l}")[:, k * 8:(k + 1) * 8],
                                     zb[:, k * 128:(k + 1) * 128],
                                     start=(k == 0), stop=(k == 1))
                es_sb = sbC.tile([8, 128], BF16, tag="essb")
                ed_sb = sbC.tile([8, 128], BF16, tag="essb")
                nc.scalar.copy(out=es_sb[:, :], in_=esed[:, 0:128])
                nc.scalar.copy(out=ed_sb[:, :], in_=esed[:, 128:256])

                # dense1[s, (h,d)] = es[h,s] + ed[h,d]:
                #   es part: blockones spreads es rows across column blocks;
                #   ed part: blockones block h is the row-h selector, so
                #   sel_h.T @ ed broadcasts ed row h down all partitions.
                dense1 = psD.tile([128, 1024], F32, tag="dense")
                for h in range(NH):
                    sl = slice(h * 128, (h + 1) * 128)
                    nc.tensor.matmul(dense1[:, sl], es_sb[:, :],
                                     W("blockones")[:, sl], start=True, stop=False)
                    nc.tensor.matmul(dense1[:, sl], W("blockones")[:, sl],
                                     ed_sb[:, :], start=False, stop=True)
                # leaky_relu(x) = max(x, 0.2x), then exp, then * cnt
                # (processed in 512-col halves so ACT/DVE pipeline)
                d1c = sbC.tile([128, 1024], F32, tag="d1c", bufs=1)
                e1 = sbC.tile([128, 1024], F32, tag="e1", bufs=1)
                w1 = sbC.tile([128, 1024], F32, tag="w1", bufs=1)
                wc = sbC.tile([128, 1024], BF16, tag="wc", bufs=1)
                for nchunk in range(2):
                    sl = slice(nchunk * 512, (nchunk + 1) * 512)
                    nc.scalar.activation(out=d1c[:, sl], in_=dense1[:, sl],
                                         func=ACTF.Identity)
                    nc.vector.scalar_tensor_tensor(out=e1[:, sl], in0=d1c[:, sl],
                                                   scalar=0.2, in1=d1c[:, sl],
                                                   op0=ALU.mult, op1=ALU.max)
                    nc.scalar.activation(out=w1[:, sl], in_=e1[:, sl],
                                         func=ACTF.Exp)
                    nc.vector.tensor_mul(
                        out=wc[:, sl].rearrange("p (h d) -> p h d", h=4),
                        in0=w1[:, sl].rearrange("p (h d) -> p h d", h=4),
                        in1=F("cnt_sd").unsqueeze(1).broadcast_to([128, 4, 128]))
                denexp = psS.tile([128, 256], F32, tag="denexp", bufs=1)
                for h in range(NH):
                    nc.tensor.matmul(
                        denexp[32 * (h % 4):32 * (h % 4) + 32,
                               128 * (h // 4):128 * (h // 4) + 128],
                        W("ones32"), wc[:, h * 128:(h + 1) * 128],
                        start=True, stop=True,
                        tile_position=(0, 32 * (h % 4)))
                dxs = sbC.tile([128, 256], F32, tag="dxs")
                nc.scalar.copy(out=dxs[:, :], in_=denexp[:, :])
                denx = sbC.tile([128, 256], F32, tag="denx")
                nc.vector.reciprocal_approx_fast(out=denx[:, :], in_=dxs[:, :])
                # zT + aggregation
                zT = sbC.tile([128, 256], BF16, tag="zT")
                for m in range(2):
                    tp = psS.tile([128, 128], BF16, tag="ps", bufs=2)
                    nc.tensor.transpose(tp[:, :], zb[:, m * 128:(m + 1) * 128],
                                        W("ident_b"))
                    nc.scalar.copy(out=zT[:, m * 128:(m + 1) * 128], in_=tp[:, :])
                agg = psS.tile([128, 256], F32, tag="agg", bufs=1)
                for h in range(NH):
                    nc.tensor.matmul(
                        agg[32 * (h % 4):32 * (h % 4) + 32,
                            128 * (h // 4):128 * (h // 4) + 128],
                        zT[:, h * 32:(h + 1) * 32],
                        wc[:, h * 128:(h + 1) * 128],
                        start=True, stop=True,
                        tile_position=(0, 32 * (h % 4)))
                # combine + LayerNorm + relu
                xln = sbC.tile([128, 256], F32, tag="xln")
                for m in range(2):
                    tmp = sbC.tile([128, 128], F32, tag="tmp")
                    nc.vector.tensor_mul(out=tmp[:, :],
                                         in0=agg[:, m * 128:(m + 1) * 128],
                                         in1=denx[:, m * 128:(m + 1) * 128])
                    nc.vector.scalar_tensor_tensor(
                        out=xln[:, m * 128:(m + 1) * 128], in0=tmp[:, :],
                        scalar=F(f"gatb{l}")[:, m:m + 1],
                        in1=hres[:, m * 128:(m + 1) * 128],
                        op0=ALU.add, op1=ALU.add)
                xsq = sbC.tile([128, 256], F32, tag="xsq")
                nc.vector.tensor_mul(out=xsq[:, :], in0=xln[:, :], in1=xln[:, :])
                stats = psS.tile([1, 256], F32, tag="stats", bufs=1)
                for k in range(2):
                    nc.tensor.matmul(stats[:, 0:128], F("onescol_f", cols=1),
                                     xln[:, k * 128:(k + 1) * 128],
                                     start=(k == 0), stop=(k == 1))
                for k in range(2):
                    nc.tensor.matmul(stats[:, 128:256], F("onescol_f", cols=1),
                                     xsq[:, k * 128:(k + 1) * 128],
                                     start=(k == 0), stop=(k == 1))
                stsb = sbC.tile([1, 256], F32, tag="stsb")
                nc.vector.tensor_scalar_mul(out=stsb[:, :], in0=stats[:, :],
                                            scalar1=1.0 / 256.0)
                musq = sbC.tile([1, 128], F32, tag="musq")
                nc.vector.tensor_mul(out=musq[:, :], in0=stsb[:, 0:128],
                                     in1=stsb[:, 0:128])
                var = sbC.tile([1, 128], F32, tag="var")
                nc.vector.tensor_sub(out=var[:, :], in0=stsb[:, 128:256],
                                     in1=musq[:, :])
                sd = sbC.tile([1, 128], F32, tag="sd")
                nc.scalar.activation(out=sd[:, :], in_=var[:, :], func=ACTF.Sqrt,
                                     bias=eps_t[:, :])
                rstd = sbC.tile([1, 128], F32, tag="rstd")
                nc.vector.reciprocal_approx_fast(out=rstd[:, :], in_=sd[:, :])
                musd = psS.tile([128, 256], F32, tag="stats", bufs=1)
                nc.tensor.matmul(musd[:, 0:128], F("onesrow_f"), stsb[:, 0:128],
                                 start=True, stop=True)
                nc.tensor.matmul(musd[:, 128:256], F("onesrow_f"), rstd[:, :],
                                 start=True, stop=True)
                hb_next = act.tile([128, 256], BF16, tag=f"hb{l + 1}",
                                   name=f"hbn{l}") if not last else None
                for m in range(2):
                    df = sbC.tile([128, 128], F32, tag="df")
                    nc.vector.tensor_sub(out=df[:, :],
                                         in0=xln[:, m * 128:(m + 1) * 128],
                                         in1=musd[:, 0:128])
                    dn = sbC.tile([128, 128], F32, tag="dn")
                    nc.vector.tensor_mul(out=dn[:, :], in0=df[:, :],
                                         in1=musd[:, 128:256])
                    dst = h3f if last else hb_next
                    nc.scalar.activation(out=dst[:, m * 128:(m + 1) * 128],
                                         in_=dn[:, :], func=ACTF.Relu,
                                         bias=F(f"lnb{l}")[:, m:m + 1],
                                         scale=F(f"lng{l}")[:, m:m + 1])
                if dbg:
                    hld = sbC.tile([128, 128], F32, tag="hld")
                    for m in range(2):
                        src = h3f if last else hb_next
                        nc.scalar.copy(out=hld[:, :],
                                       in_=src[:, m * 128:(m + 1) * 128])
                        nc.sync.dma_start(out=dbg[f"hl{l}"].ap()[m], in_=hld[:, :])
                if not last:
                    hb = hb_next

        if stop_after == "C":
            raise _StopEmit

        # ============ Phase D: pool/conv head -> comb output =============
        with tc.tile_pool(name="psE", bufs=1, space="PSUM") as psE, \
             tc.tile_pool(name="sbE", bufs=2) as sbE:
            hmean = sbE.tile([128, 2], F32, tag="hmean")
            hmax = sbE.tile([128, 2], F32, tag="hmax")
            for m in range(2):
                nc.vector.reduce_sum(out=hmean[:, m:m + 1],
                                     in_=h3f[:, m * 128:(m + 1) * 128], axis=AX.X)
                nc.vector.reduce_max(out=hmax[:, m:m + 1],
                                     in_=h3f[:, m * 128:(m + 1) * 128], axis=AX.X)
            catb = sbE.tile([128, 4], BF16, tag="catb")
            for m in range(2):
                nc.scalar.activation(out=catb[:, m:m + 1], in_=hmean[:, m:m + 1],
                                     func=ACTF.Identity, scale=1.0 / T)
                nc.scalar.copy(out=catb[:, 2 + m:3 + m], in_=hmax[:, m:m + 1])
            cat4 = sbE.tile([128, 4], F32, tag="cat4")
            tf_b = sbE.tile([128, 2], BF16, tag="tf_b")
            for m in range(2):
                ps = psE.tile([128, 1], F32, tag="v", bufs=2)
                for k in range(4):
                    nc.tensor.matmul(
                        ps[:, :],
                        W("tpwT")[:, k * 256 + m * 128: k * 256 + (m + 1) * 128],
                        catb[:, k:k + 1], start=(k == 0), stop=(k == 3))
                nc.scalar.activation(out=cat4[:, m:m + 1], in_=ps[:, :],
                                     func=ACTF.Relu, bias=F("tpb")[:, m:m + 1])
            nc.vector.tensor_copy(out=tf_b[:, :], in_=cat4[:, 0:2])
            c1_b = sbE.tile([128, 2], BF16, tag="c1_b")
            for m in range(2):
                ps = psE.tile([128, 1], F32, tag="v", bufs=2)
                for k in range(2):
                    nc.tensor.matmul(
                        ps[:, :],
                        W("c1wT")[:, k * 256 + m * 128: k * 256 + (m + 1) * 128],
                        tf_b[:, k:k + 1], start=(k == 0), stop=(k == 1))
                nc.scalar.activation(out=c1_b[:, m:m + 1], in_=ps[:, :],
                                     func=ACTF.Relu, bias=F("c1t")[:, m:m + 1],
                                     scale=F("c1s")[:, m:m + 1])
            for m in range(2):
                ps = psE.tile([128, 1], F32, tag="v", bufs=2)
                for k in range(2):
                    nc.tensor.matmul(
                        ps[:, :],
                        W("c2wT")[:, k * 256 + m * 128: k * 256 + (m + 1) * 128],
                        c1_b[:, k:k + 1], start=(k == 0), stop=(k == 1))
                nc.scalar.activation(out=cat4[:, 2 + m:3 + m], in_=ps[:, :],
                                     func=ACTF.Relu, bias=F("c2t")[:, m:m + 1],
                                     scale=F("c2s")[:, m:m + 1])
            ctp = psE.tile([4, 128], F32, tag="ctp", bufs=1)
            nc.tensor.transpose(ctp[:, :], cat4[:, :], F("ident_f"))
            comb_sb = sbE.tile([4, 128], F32, tag="comb_sb")
            nc.scalar.copy(out=comb_sb[:, :], in_=ctp[:, :])
            nc.sync.dma_start(out=comb_d.ap(), in_=comb_sb[:, :])
    except _StopEmit:
        pass
    act.release()
    wp.release()


# ---------------------------------------------------------------------------
# Entry point
# ---------------------------------------------------------------------------

_NC_CACHE = {}


def get_program(debug=False):
    key = bool(debug)
    if key not in _NC_CACHE:
        _NC_CACHE[key] = build_program(debug=key)
    return _NC_CACHE[key]


def run_device(in_maps, debug=False, **kwargs):
    nc = get_program(debug=debug)
    return bass_utils.run_bass_kernel_spmd(nc, in_maps,
                                           core_ids=list(range(N_CORES)), **kwargs)


def kernel(feats, edge_index, batch_idx, params):
    del batch_idx  # all-zero; one graph per sequence
    in_maps = prep_in_maps(feats, edge_index, params)
    res = run_device(in_maps)
    comb = np.concatenate([np.asarray(res.results[b]["comb"], np.float32)
                           for b in range(B)], axis=0)
    return host_epilogue(comb, params)
